# revision 3
# baseline (speedup 1.0000x reference)
"""KroneLinear Trainium2 kernel.

Math (per sample n): Y_n = W_a @ X_n @ W_b^T + B
  where X_n = x[n].reshape(128,128) (j outer, k inner),
        Y_n = y[n].reshape(128,128) (a outer, o inner), B = bias.reshape(128,128).

Implementation (per NeuronCore, pure data-parallel over batch, 512 samples/core):
  mm1: T_n = X_n^T @ W_a^T   (lhsT = X_n natural layout, rhs = W_a^T)  -> [k, a] in PSUM
  mm2: Y_n = T_n^T @ W_b^T   (lhsT = T_n copied to SBUF, rhs = W_b^T)  -> [a, o] in PSUM
  bias add on vector engine, then DMA out. No transposes anywhere: the
  intrinsic Kronecker transpose is absorbed by using the per-sample data as the
  stationary (pre-transposed) matmul operand both times.

Weights are pre-transposed on the host (tiny 128x128) and passed as extra
kernel parameters; bias is pre-tiled to [128, G*128] so one tensor_add
covers a whole G-sample group.
"""

import sys
import os

if "/opt/trn_rl_repo" not in sys.path:
    sys.path.insert(0, "/opt/trn_rl_repo")

import numpy as np

N_CORES = 8
N_FULL = 4096
SHARD = N_FULL // N_CORES  # 512 samples per core
P = 128
D = P * P  # 16384
G = 4  # samples per DMA/compute group (PSUM bank = 512 fp32 = 4 samples)

_cache = {}


def _build_nc():
    import concourse.tile as tile
    from concourse import bacc, mybir

    f32 = mybir.dt.float32
    nc = bacc.Bacc()

    x = nc.declare_dram_parameter("x", [SHARD, D], f32, isOutput=False)
    wa_t = nc.declare_dram_parameter("wa_t", [P, P], f32, isOutput=False)
    wb_t = nc.declare_dram_parameter("wb_t", [P, P], f32, isOutput=False)
    bias4 = nc.declare_dram_parameter("bias4", [P, G, P], f32, isOutput=False)
    y = nc.declare_dram_parameter("y", [SHARD, D], f32, isOutput=True)

    ngroups = SHARD // G

    with tile.TileContext(nc) as tc:
        with (
            tc.tile_pool(name="consts", bufs=1) as consts,
            tc.tile_pool(name="xin", bufs=4) as xpool,
            tc.tile_pool(name="tsb", bufs=3) as tpool,
            tc.tile_pool(name="outp", bufs=4) as opool,
            tc.tile_pool(name="ps1", bufs=3, space="PSUM") as ps1,
            tc.tile_pool(name="ps2", bufs=3, space="PSUM") as ps2,
        ):
            wa_sb = consts.tile([P, P], f32)
            nc.sync.dma_start(wa_sb[:], wa_t[:, :])
            wb_sb = consts.tile([P, P], f32)
            nc.sync.dma_start(wb_sb[:], wb_t[:, :])
            bias_sb = consts.tile([P, G, P], f32)
            nc.sync.dma_start(bias_sb[:], bias4[:, :, :])

            # x[n, j*128+k] viewed per group as [j(part), s, k]
            xv = x.rearrange("(g s) (j k) -> g j s k", s=G, j=P)
            # y[n, a*128+o] viewed per group as [a(part), s, o]
            yv = y.rearrange("(g s) (a o) -> g a s o", s=G, a=P)

            for g in range(ngroups):
                xt = xpool.tile([P, G, P], f32)
                nc.sync.dma_start(xt[:], xv[g])

                t_ps = ps1.tile([P, G, P], f32)
                for s in range(G):
                    # T = X^T @ Wa^T : [k, a]
                    nc.tensor.matmul(
                        t_ps[:, s, :], xt[:, s, :], wa_sb[:], start=True, stop=True
                    )
                t_sb = tpool.tile([P, G, P], f32)
                nc.scalar.copy(t_sb[:], t_ps[:])

                y_ps = ps2.tile([P, G, P], f32)
                for s in range(G):
                    # Y = T^T @ Wb^T : [a, o]
                    nc.tensor.matmul(
                        y_ps[:, s, :], t_sb[:, s, :], wb_sb[:], start=True, stop=True
                    )
                ot = opool.tile([P, G, P], f32)
                nc.vector.tensor_add(ot[:], y_ps[:], bias_sb[:])
                nc.sync.dma_start(yv[g], ot[:])

    if not nc.is_finalized():
        nc.finalize()
    return nc


PROFILE = False
LAST_RESULT = None


def kernel(x, weight_a, weight_b, bias):
    global LAST_RESULT
    from concourse.bass_utils import run_bass_kernel_spmd

    if "nc" not in _cache:
        _cache["nc"] = _build_nc()
    nc = _cache["nc"]

    x = np.asarray(x, dtype=np.float32)
    wa_t = np.ascontiguousarray(np.asarray(weight_a, np.float32).T)
    wb_t = np.ascontiguousarray(np.asarray(weight_b, np.float32).T)
    b2 = np.asarray(bias, np.float32).reshape(P, P)
    bias4 = np.ascontiguousarray(np.broadcast_to(b2[:, None, :], (P, G, P)))

    in_maps = [
        {
            "x": np.ascontiguousarray(x[c * SHARD : (c + 1) * SHARD]),
            "wa_t": wa_t,
            "wb_t": wb_t,
            "bias4": bias4,
        }
        for c in range(N_CORES)
    ]
    res = run_bass_kernel_spmd(
        nc, in_maps, core_ids=list(range(N_CORES)), trace=PROFILE
    )
    LAST_RESULT = res
    return np.concatenate([res.results[c]["y"] for c in range(N_CORES)], axis=0)


# revision 5
# speedup vs baseline: 1.7460x; 1.7460x over previous
"""KroneLinear Trainium2 kernel.

Math (per sample n): Y_n = W_a @ X_n @ W_b^T + B
  where X_n = x[n].reshape(128,128) (j outer, k inner),
        Y_n = y[n].reshape(128,128) (a outer, o inner), B = bias.reshape(128,128).

Implementation (per NeuronCore, pure data-parallel over batch, 512 samples/core):
  mm1: T_n = X_n^T @ W_a^T   (lhsT = X_n natural layout, rhs = W_a^T)  -> [k, a] in PSUM
  mm2: Y_n = T_n^T @ W_b^T   (lhsT = T_n copied to SBUF, rhs = W_b^T)  -> [a, o] in PSUM
  bias add on vector engine, then DMA out. No transposes anywhere: the
  intrinsic Kronecker transpose is absorbed by using the per-sample data as the
  stationary (pre-transposed) matmul operand both times.

Weights are pre-transposed on the host (tiny 128x128) and passed as extra
kernel parameters; bias is pre-tiled to [128, G*128] so one tensor_add
covers a whole G-sample group.
"""

import sys
import os

if "/opt/trn_rl_repo" not in sys.path:
    sys.path.insert(0, "/opt/trn_rl_repo")

import numpy as np

N_CORES = 8
N_FULL = 4096
SHARD = N_FULL // N_CORES  # 512 samples per core
P = 128
D = P * P  # 16384
G = 4  # samples per DMA/compute group (PSUM bank = 512 fp32 = 4 samples)

_cache = {}


def _build_nc(reps=1):
    import concourse.tile as tile
    from concourse import bacc, mybir

    f32 = mybir.dt.float32
    nc = bacc.Bacc()

    x = nc.declare_dram_parameter("x", [SHARD, D], f32, isOutput=False)
    wa_t = nc.declare_dram_parameter("wa_t", [P, P], f32, isOutput=False)
    wb_t = nc.declare_dram_parameter("wb_t", [P, P], f32, isOutput=False)
    bias4 = nc.declare_dram_parameter("bias4", [P, G, P], f32, isOutput=False)
    y = nc.declare_dram_parameter("y", [SHARD, D], f32, isOutput=True)

    ngroups = SHARD // G

    with tile.TileContext(nc) as tc:
        with (
            tc.tile_pool(name="consts", bufs=1) as consts,
            tc.tile_pool(name="xin", bufs=4) as xpool,
            tc.tile_pool(name="tsb", bufs=3) as tpool,
            tc.tile_pool(name="outp", bufs=4) as opool,
            tc.tile_pool(name="ps1", bufs=3, space="PSUM") as ps1,
            tc.tile_pool(name="ps2", bufs=3, space="PSUM") as ps2,
        ):
            wa_sb = consts.tile([P, P], f32)
            nc.sync.dma_start(wa_sb[:], wa_t[:, :])
            wb_sb = consts.tile([P, P], f32)
            nc.sync.dma_start(wb_sb[:], wb_t[:, :])
            bias_sb = consts.tile([P, G, P], f32)
            nc.sync.dma_start(bias_sb[:], bias4[:, :, :])

            # x[n, j*128+k] viewed per group as [j(part), s, k]
            xv = x.rearrange("(g s) (j k) -> g j s k", s=G, j=P)
            # y[n, a*128+o] viewed per group as [a(part), s, o]
            yv = y.rearrange("(g s) (a o) -> g a s o", s=G, a=P)

            for g in [g for _ in range(reps) for g in range(ngroups)]:
                xt = xpool.tile([P, G, P], f32)
                nc.sync.dma_start(xt[:], xv[g])

                t_ps = ps1.tile([P, G, P], f32)
                for s in range(G):
                    # T = X^T @ Wa^T : [k, a]
                    nc.tensor.matmul(
                        t_ps[:, s, :], xt[:, s, :], wa_sb[:], start=True, stop=True
                    )
                t_sb = tpool.tile([P, G, P], f32)
                nc.scalar.copy(t_sb[:], t_ps[:])

                y_ps = ps2.tile([P, G, P], f32)
                for s in range(G):
                    # Y = T^T @ Wb^T : [a, o]
                    nc.tensor.matmul(
                        y_ps[:, s, :], t_sb[:, s, :], wb_sb[:], start=True, stop=True
                    )
                ot = opool.tile([P, G, P], f32)
                nc.vector.tensor_add(ot[:], y_ps[:], bias_sb[:])
                nc.sync.dma_start(yv[g], ot[:])

    if not nc.is_finalized():
        nc.finalize()
    return nc


PROFILE = False
LAST_RESULT = None


def kernel(x, weight_a, weight_b, bias):
    global LAST_RESULT
    from concourse.bass_utils import run_bass_kernel_spmd

    if "nc" not in _cache:
        _cache["nc"] = _build_nc()
    nc = _cache["nc"]

    x = np.asarray(x, dtype=np.float32)
    wa_t = np.ascontiguousarray(np.asarray(weight_a, np.float32).T)
    wb_t = np.ascontiguousarray(np.asarray(weight_b, np.float32).T)
    b2 = np.asarray(bias, np.float32).reshape(P, P)
    bias4 = np.ascontiguousarray(np.broadcast_to(b2[:, None, :], (P, G, P)))

    in_maps = [
        {
            "x": np.ascontiguousarray(x[c * SHARD : (c + 1) * SHARD]),
            "wa_t": wa_t,
            "wb_t": wb_t,
            "bias4": bias4,
        }
        for c in range(N_CORES)
    ]
    res = run_bass_kernel_spmd(
        nc, in_maps, core_ids=list(range(N_CORES)), trace=PROFILE
    )
    LAST_RESULT = res
    return np.concatenate([res.results[c]["y"] for c in range(N_CORES)], axis=0)
